# revision 28
# baseline (speedup 1.0000x reference)
"""Self-attention (SAGAN-style) Trainium2 kernel.

Reference computation (per batch sample):
    theta = w_theta @ x            # [32, 4096]
    phi   = pool2x2(w_phi @ x)     # [32, 1024]
    g     = pool2x2(w_g @ x)       # [128, 1024]
    beta  = softmax(theta.T @ phi, axis=-1)   # [4096, 1024]
    attn  = g @ beta.T             # [128, 4096]
    out   = gamma * (w_o @ attn) + x

Sharding: data-parallel over batch; B=16 over 8 cores -> 2 samples/core.

Schedule (the critical property is a gapless PE stream: TRN2's tensor
engine only reaches full clock after ~3us of continuous execution and
drops to half speed after every stall):
  - the two samples alternate at quarter granularity; sample 1's
    projections/transposes are interleaved into sample 0's early score
    phases as PE filler.
  - attention + softmax-denominator units trail their score quarter by
    one phase, out-projection units by two, so every PE instruction's
    inputs are produced ~8us before the PE reaches it.
  - exp tiles and denominator partials are 2-quarter rings in SBUF; the
    score PSUM ring is dedicated to score tiles only (projection tiles
    live in the unit ring) so PSUM recycling never couples the PE to
    ScalarE's exp cadence.

Compute mapping:
  - scores run 2-way quadrant-packed (theta/phi duplicated by the
    projection into [th th ph ph] rows; even/odd m-chunks issue at
    tile_position (0,0)/(32,0) reading disjoint moving partitions).
  - softmax denominator: DVE pairwise+tree bf16 adds of the exp tiles,
    then a single K=128 ones-matmul per chunk broadcasts the per-column
    sum across partitions (vs. 8 matmuls if reduced on the PE).
  - residual: identity-matmul accumulated into the out-projection PSUM
    group; evacuation split between ScalarE and DVE copies.
  - exps + theta/output evacuation on ScalarE; pools (single fused 2x2
    reduce straight from PSUM), denominator adds, reciprocal, attention
    normalize on DVE; x casting-loads (f32->bf16) on the GPSIMD SWDGE
    queue; weights pre-cast to bf16 on host and loaded via sync HWDGE;
    output stores on SP.
"""

import numpy as np

import concourse.bacc as bacc
import concourse.mybir as mybir
from concourse import tile
from concourse.bass_utils import run_bass_kernel_spmd
from concourse.alu_op_type import AluOpType

F32 = mybir.dt.float32
BF16 = mybir.dt.bfloat16
EXP = mybir.ActivationFunctionType.Exp

B, C, H, W = 16, 256, 64, 64
N = H * W            # 4096
M = N // 4           # 1024
C8 = C // 8          # 32
C2 = C // 2          # 128
NCORES = 8
BPC = B // NCORES    # 2 samples per core
MC = M // 128        # 8 m-chunks


def build_kernel():
    nc = bacc.Bacc("TRN2", target_bir_lowering=False, debug=False)

    x_d = nc.declare_dram_parameter("x", [BPC, C, N], F32, isOutput=False)
    wq_d = nc.declare_dram_parameter("wq", [2, 128, 128], BF16, isOutput=False)
    wg_d = nc.declare_dram_parameter("wg", [2, 128, C2], BF16, isOutput=False)
    wo_d = nc.declare_dram_parameter("wo", [2, C2, 128], BF16, isOutput=False)
    id_d = nc.declare_dram_parameter("ident", [128, 128], BF16, isOutput=False)
    out_d = nc.declare_dram_parameter("out", [BPC, C, N], F32, isOutput=True)

    with tile.TileContext(nc) as tc:
        with (
            tc.tile_pool(name="const", bufs=1) as constp,
            tc.tile_pool(name="data", bufs=1) as datap,
            tc.tile_pool(name="small", bufs=3) as smallp,
            tc.tile_pool(name="outs", bufs=4) as outp,
            tc.tile_pool(name="ps_big", bufs=2, space="PSUM") as psb,
            tc.tile_pool(name="ps_u", bufs=4, space="PSUM") as psu,
        ):
            # ---- weights: pre-cast bf16 on host, loaded via sync HWDGE
            # (runs in parallel with the x casting loads on the SWDGE queue)
            wq, wg, wo = [], [], []
            for cc in range(2):
                t = constp.tile([128, 128], BF16, tag=f"wq{cc}", name=f"wq{cc}")
                nc.sync.dma_start(t[:], wq_d[cc])
                wq.append(t)
                t = constp.tile([128, C2], BF16, tag=f"wg{cc}", name=f"wg{cc}")
                nc.sync.dma_start(t[:], wg_d[cc])
                wg.append(t)
            for oc in range(2):
                t = constp.tile([C2, 128], BF16, tag=f"wo{oc}", name=f"wo{oc}")
                nc.sync.dma_start(t[:], wo_d[oc])
                wo.append(t)
            id_b = constp.tile([128, 128], BF16, tag="id_b", name="id_b")
            nc.sync.dma_start(id_b[:], id_d[:])
            ones = constp.tile([128, 128], BF16, tag="ones", name="ones")
            nc.gpsimd.memset(ones[:], 1.0)

            # ---- x: bf16 casting loads, resident for residual ----
            xb = [dict() for _ in range(BPC)]
            for b in range(BPC):
                for half in range(2):
                    for cc in range(2):
                        t = datap.tile([128, 2048], BF16, tag=f"xb{b}{cc}{half}",
                                       name=f"xb{b}_{cc}_{half}")
                        xb[b][(cc, half)] = t
            for b in range(BPC):
                for half in range(2):
                    for p0 in range(0, 2048, 512):
                        for cc in range(2):
                            src = slice(half * 2048 + p0, half * 2048 + p0 + 512)
                            nc.gpsimd.dma_start(
                                xb[b][(cc, half)][:, p0:p0 + 512],
                                x_d[b, cc * 128:(cc + 1) * 128, src])

            # ---- per-sample state ----
            st = []
            for b in range(BPC):
                s = dict(aps={}, at={})
                s["th2"] = datap.tile([64, N], BF16, tag=f"th2{b}",
                                      name=f"th2_{b}")
                s["ph2"] = datap.tile([64, M], BF16, tag=f"ph2{b}",
                                      name=f"ph2_{b}")
                s["gp"] = datap.tile([C2, M], BF16, tag=f"gp{b}", name=f"gp_{b}")
                # ets / psum partial tiles are 2-quarter rings along free axis
                s["ets"] = [datap.tile([128, 2048], BF16, tag=f"ets{b}{mc}",
                                       name=f"ets{b}_{mc}") for mc in range(MC)]
                s["psum4"] = [datap.tile([128, 2048], BF16, tag=f"ps4_{b}{j}",
                                         name=f"ps4_{b}_{j}") for j in range(4)]
                s["gts"] = [None] * MC
                st.append(s)

            # ---------------- emitters ----------------
            def proj_chunk(b, i):
                half, off = i // 4, (i % 4) * 512
                s = st[b]
                xs = [xb[b][(cc, half)][:, off:off + 512] for cc in range(2)]
                # separate 1-bank tiles in the unit ring: the big ring stays
                # dedicated to score pairs (avoids ACT-gated ring stalls)
                gd = psu.tile([128, 512], F32, tag="u", name=f"pg{b}_{i}")
                qd = psu.tile([128, 512], F32, tag="u", name=f"pq{b}_{i}")
                for cc in range(2):
                    nc.tensor.matmul(gd[:], wg[cc][:], xs[cc],
                                     start=(cc == 0), stop=(cc == 1))
                # q-proj rows: [th th ph ph] duplicated for quadrant packing
                for cc in range(2):
                    nc.tensor.matmul(qd[:], wq[cc][:], xs[cc],
                                     start=(cc == 0), stop=(cc == 1))
                sl = slice(i * 512, (i + 1) * 512)
                nc.scalar.copy(s["th2"][:, sl], qd[0:64, :])
                # fused 2x2 maxpools: single DVE reduce over (hb, two) axes
                msl = slice(i * 128, (i + 1) * 128)
                pv = qd[64:128, :].rearrange(
                    "p (h2 hb w2 two) -> p h2 w2 hb two", h2=4, hb=2, w2=32, two=2)
                nc.vector.tensor_reduce(
                    s["ph2"][:, msl].rearrange("p (h2 w2) -> p h2 w2", h2=4, w2=32),
                    pv, mybir.AxisListType.XY, AluOpType.max)
                gv = gd[:].rearrange(
                    "p (h2 hb w2 two) -> p h2 w2 hb two", h2=4, hb=2, w2=32, two=2)
                nc.vector.tensor_reduce(
                    s["gp"][:, msl].rearrange("p (h2 w2) -> p h2 w2", h2=4, w2=32),
                    gv, mybir.AxisListType.XY, AluOpType.max)

            def tp_chunk(b, mc):
                s = st[b]
                tp = psb.tile([128, 128], BF16, tag="big", name=f"tp{b}_{mc}")
                nc.tensor.transpose(tp[:], s["gp"][:, mc * 128:(mc + 1) * 128],
                                    id_b[:])
                gt = datap.tile([128, 128], BF16, tag=f"gt{b}{mc}",
                                name=f"gt{b}_{mc}")
                nc.vector.tensor_copy(gt[:], tp[:])
                s["gts"][mc] = gt

            def score_pair(b, qt, r):
                """Quadrant-packed scores for m-chunks (2r, 2r+1)."""
                s = st[b]
                ring = (qt % 2) * 1024
                qsl = slice(ring, ring + 1024)
                mca, mcb = 2 * r, 2 * r + 1
                spa = psb.tile([128, 1024], F32, tag="big", name=f"sa{b}_{qt}_{r}")
                spb = psb.tile([128, 1024], F32, tag="big", name=f"sb{b}_{qt}_{r}")
                for q, sp_t, mc in ((0, spa, mca), (32, spb, mcb)):
                    for hf in range(2):
                        nsl = slice(qt * 1024 + hf * 512,
                                    qt * 1024 + (hf + 1) * 512)
                        osl = slice(hf * 512, (hf + 1) * 512)
                        nc.tensor.matmul(sp_t[:, osl],
                                         s["ph2"][q:q + 32,
                                                  mc * 128:(mc + 1) * 128],
                                         s["th2"][q:q + 32, nsl],
                                         start=True, stop=True,
                                         tile_position=(q, 0))
                nc.scalar.activation(s["ets"][mca][:, qsl], spa[:], EXP)
                nc.scalar.activation(s["ets"][mcb][:, qsl], spb[:], EXP)
                # denominator partials: pairwise adds + in-place tree on DVE
                ps4 = s["psum4"]
                nc.vector.tensor_tensor(ps4[r][:, qsl], s["ets"][mca][:, qsl],
                                        s["ets"][mcb][:, qsl], AluOpType.add)
                if r == 3:
                    nc.vector.tensor_tensor(ps4[0][:, qsl], ps4[0][:, qsl],
                                            ps4[1][:, qsl], AluOpType.add)
                    nc.vector.tensor_tensor(ps4[2][:, qsl], ps4[2][:, qsl],
                                            ps4[3][:, qsl], AluOpType.add)
                    nc.vector.tensor_tensor(ps4[0][:, qsl], ps4[0][:, qsl],
                                            ps4[2][:, qsl], AluOpType.add)

            def unit_attn2(b, i0):
                s = st[b]
                ring = ((i0 // 2) % 2) * 1024
                rs = [slice(ring + (i % 2) * 512, ring + (i % 2) * 512 + 512)
                      for i in (i0, i0 + 1)]
                ap2 = []
                for i in (i0, i0 + 1):
                    aps = psu.tile([128, 512], F32, tag="u", name=f"aps{b}_{i}")
                    s["aps"][i] = aps
                    ap2.append(aps)
                for mc in range(MC):
                    for j in range(2):
                        nc.tensor.matmul(ap2[j][:], s["gts"][mc][:],
                                         s["ets"][mc][:, rs[j]],
                                         start=(mc == 0), stop=(mc == MC - 1),
                                         skip_group_check=True)

            def unit_den2(b, i0):
                s = st[b]
                ring = ((i0 // 2) % 2) * 1024
                for i in (i0, i0 + 1):
                    rsl = slice(ring + (i % 2) * 512, ring + (i % 2) * 512 + 512)
                    dps = psu.tile([128, 512], F32, tag="u", name=f"dps{b}_{i}")
                    nc.tensor.matmul(dps[:], ones[:], s["psum4"][0][:, rsl],
                                     start=True, stop=True)
                    rec = smallp.tile([128, 512], F32, tag="rec",
                                      name=f"rec{b}_{i}")
                    nc.vector.reciprocal_approx_fast(rec[:], dps[:])
                    at = smallp.tile([128, 512], BF16, tag="at", name=f"at{b}_{i}")
                    nc.vector.scalar_tensor_tensor(
                        at[:], s["aps"][i][:], 1.0, rec[:],
                        AluOpType.bypass, AluOpType.mult)
                    s["at"][i] = at

            def unit_out(b, i):
                s = st[b]
                nsl = slice(i * 512, (i + 1) * 512)
                half, off = i // 4, (i % 4) * 512
                for oc in range(2):
                    op = psu.tile([128, 512], F32, tag="u", name=f"op{b}_{i}_{oc}")
                    nc.tensor.matmul(op[:], id_b[:],
                                     xb[b][(oc, half)][:, off:off + 512],
                                     start=True, stop=False,
                                     skip_group_check=True)
                    nc.tensor.matmul(op[:], wo[oc][:], s["at"][i][:],
                                     start=False, stop=True,
                                     skip_group_check=True)
                    osb = outp.tile([128, 512], F32, tag="osb",
                                    name=f"osb{b}_{i}_{oc}")
                    if oc == 0:
                        nc.scalar.copy(osb[:], op[:])
                    else:
                        nc.vector.tensor_copy(osb[:], op[:])
                    nc.sync.dma_start(out_d[b, oc * 128:(oc + 1) * 128, nsl],
                                      osb[:])

            # ---------------- emission schedule ----------------
            # Phase order interleaves the two samples and overlaps sample 1's
            # projections/transposes into sample 0's early score phases; units
            # trail their quarter by one phase (FIFO).
            pending = []

            def pop(k):
                for _ in range(k):
                    if pending:
                        pending.pop(0)()

            oq = []

            def queue_units(b, qt):
                # out units trail two phases (normalized inputs long-ready)
                # and pop first, covering the exp tail before attn/den pop
                i0 = 2 * qt
                pending.extend(oq)
                oq.clear()
                pending.append(lambda: unit_attn2(b, i0))
                pending.append(lambda: unit_den2(b, i0))
                oq.append(lambda: unit_out(b, i0))
                oq.append(lambda: unit_out(b, i0 + 1))

            # opening: proj.b0 with qt0.b0's score pairs interleaved (pair r
            # needs only proj chunks <= 2r+1), transposes at the end
            for r in range(4):
                proj_chunk(0, 2 * r)
                proj_chunk(0, 2 * r + 1)
                score_pair(0, 0, r)
            for mc in range(MC):
                tp_chunk(0, mc)
            queue_units(0, 0)

            # filler work per phase: (emitted after round r as PE filler)
            fillers = {
                (0, 1): [lambda i=i: proj_chunk(1, i) for i in range(3)],
                (0, 2): [lambda i=i: proj_chunk(1, i) for i in range(3, 6)],
            }
            # (1,0)'s remaining projections must land before round r uses
            # their ph2 blocks: emit both right after r0 (front-loaded)
            pre = {
                (1, 0): [lambda i=i: proj_chunk(1, i) for i in range(6, MC)],
            }
            post = {
                (1, 0): [lambda mc=mc: tp_chunk(1, mc) for mc in range(MC)],
            }
            phase_list = [(0, 1), (0, 2), (1, 0), (0, 3), (1, 1),
                          (1, 2), (1, 3)]
            for b, qt in phase_list:
                fill = fillers.get((b, qt), [])
                fi = 0
                for r in range(4):
                    score_pair(b, qt, r)
                    if r == 0:
                        for fn in pre.get((b, qt), []):
                            fn()
                    take = (len(fill) * (r + 1)) // 4 - fi
                    for _ in range(take):
                        fill[fi]()
                        fi += 1
                    if r == 1:
                        pop(1)
                    elif r == 2:
                        pop(1)
                    elif r == 3:
                        pop(2)
                for fn in post.get((b, qt), []):
                    fn()
                queue_units(b, qt)
            pending.extend(oq)
            oq.clear()
            while pending:
                pop(1)

    nc.compile()
    return nc


_NC_CACHE = None


def _get_nc():
    global _NC_CACHE
    if _NC_CACHE is None:
        _NC_CACHE = build_kernel()
    return _NC_CACHE


def prep_inputs(x, w_theta, w_phi, w_g, w_o, gamma):
    """Host-side prep: shard x over 8 cores; transpose/scale/pack weights."""
    x = np.asarray(x, dtype=np.float32).reshape(B, C, N)
    w_theta = np.asarray(w_theta, dtype=np.float32)
    w_phi = np.asarray(w_phi, dtype=np.float32)
    w_g = np.asarray(w_g, dtype=np.float32)
    w_o = np.asarray(w_o, dtype=np.float32)
    gamma = np.float32(gamma)

    # [th th ph ph] along the output dim for quadrant-packed scores
    import ml_dtypes
    bf16 = ml_dtypes.bfloat16
    wqT = np.concatenate([w_theta.T, w_theta.T, w_phi.T, w_phi.T], axis=1)
    wq = np.ascontiguousarray(wqT.reshape(2, 128, 128)).astype(bf16)
    wgq = np.ascontiguousarray(w_g.T.reshape(2, 128, C2)).astype(bf16)
    woT = (gamma * w_o).T                                     # [128, 256]
    wo = np.ascontiguousarray(
        woT.reshape(C2, 2, 128).transpose(1, 0, 2)).astype(bf16)
    ident = np.eye(128, dtype=np.float32).astype(bf16)

    in_maps = []
    for core in range(NCORES):
        shard = np.ascontiguousarray(x[core * BPC:(core + 1) * BPC])
        in_maps.append({"x": shard, "wq": wq, "wg": wgq, "wo": wo,
                        "ident": ident})
    return in_maps


def run(inputs, trace=False, **kw):
    nc = _get_nc()
    in_maps = prep_inputs(**inputs)
    res = run_bass_kernel_spmd(nc, in_maps, core_ids=list(range(NCORES)),
                               trace=trace, **kw)
    outs = [res.results[i]["out"] for i in range(NCORES)]
    full = np.concatenate(outs, axis=0).reshape(B, C, H, W).astype(np.float32)
    return full, res


def kernel(**inputs):
    full, _ = run(inputs, trace=False)
    return full


# revision 29
# speedup vs baseline: 1.1521x; 1.1521x over previous
"""Self-attention (SAGAN-style) Trainium2 kernel.

Reference computation (per batch sample):
    theta = w_theta @ x            # [32, 4096]
    phi   = pool2x2(w_phi @ x)     # [32, 1024]
    g     = pool2x2(w_g @ x)       # [128, 1024]
    beta  = softmax(theta.T @ phi, axis=-1)   # [4096, 1024]
    attn  = g @ beta.T             # [128, 4096]
    out   = gamma * (w_o @ attn) + x

Sharding: data-parallel over batch; B=16 over 8 cores -> 2 samples/core.

Schedule (the critical property is a gapless PE stream: TRN2's tensor
engine only reaches full clock after ~3us of continuous execution and
drops to half speed after every stall):
  - the two samples alternate at quarter granularity; sample 1's
    projections/transposes are interleaved into sample 0's early score
    phases as PE filler.
  - attention + softmax-denominator units trail their score quarter by
    one phase, out-projection units by two, so every PE instruction's
    inputs are produced ~8us before the PE reaches it.
  - exp tiles and denominator partials are 2-quarter rings in SBUF; the
    score PSUM ring is dedicated to score tiles only (projection tiles
    live in the unit ring) so PSUM recycling never couples the PE to
    ScalarE's exp cadence.

Compute mapping:
  - scores run 2-way quadrant-packed (theta/phi duplicated by the
    projection into [th th ph ph] rows; even/odd m-chunks issue at
    tile_position (0,0)/(32,0) reading disjoint moving partitions).
  - softmax denominator: DVE pairwise+tree bf16 adds of the exp tiles,
    then a single K=128 ones-matmul per chunk broadcasts the per-column
    sum across partitions (vs. 8 matmuls if reduced on the PE).
  - residual: identity-matmul accumulated into the out-projection PSUM
    group; evacuation split between ScalarE and DVE copies.
  - exps + theta/output evacuation on ScalarE; pools (single fused 2x2
    reduce straight from PSUM), denominator adds, reciprocal, attention
    normalize on DVE; x casting-loads (f32->bf16) on the GPSIMD SWDGE
    queue; weights pre-cast to bf16 on host and loaded via sync HWDGE;
    output stores on SP.
"""

import numpy as np

import concourse.bacc as bacc
import concourse.mybir as mybir
from concourse import tile
from concourse.bass_utils import run_bass_kernel_spmd
from concourse.alu_op_type import AluOpType

F32 = mybir.dt.float32
BF16 = mybir.dt.bfloat16
EXP = mybir.ActivationFunctionType.Exp

B, C, H, W = 16, 256, 64, 64
N = H * W            # 4096
M = N // 4           # 1024
C8 = C // 8          # 32
C2 = C // 2          # 128
NCORES = 8
BPC = B // NCORES    # 2 samples per core
MC = M // 128        # 8 m-chunks


def build_kernel():
    nc = bacc.Bacc("TRN2", target_bir_lowering=False, debug=False)

    x_d = nc.declare_dram_parameter("x", [BPC, C, N], F32, isOutput=False)
    wq_d = nc.declare_dram_parameter("wq", [2, 128, 128], BF16, isOutput=False)
    wg_d = nc.declare_dram_parameter("wg", [2, 128, C2], BF16, isOutput=False)
    wo_d = nc.declare_dram_parameter("wo", [2, C2, 128], BF16, isOutput=False)
    id_d = nc.declare_dram_parameter("ident", [128, 128], BF16, isOutput=False)
    out_d = nc.declare_dram_parameter("out", [BPC, C, N], F32, isOutput=True)

    with tile.TileContext(nc) as tc:
        with (
            tc.tile_pool(name="const", bufs=1) as constp,
            tc.tile_pool(name="data", bufs=1) as datap,
            tc.tile_pool(name="small", bufs=3) as smallp,
            tc.tile_pool(name="outs", bufs=4) as outp,
            tc.tile_pool(name="ps_big", bufs=2, space="PSUM") as psb,
            tc.tile_pool(name="ps_u", bufs=4, space="PSUM") as psu,
        ):
            # ---- weights: pre-cast bf16 on host, loaded via sync HWDGE
            # (runs in parallel with the x casting loads on the SWDGE queue)
            wq, wg, wo = [], [], []
            for cc in range(2):
                t = constp.tile([128, 128], BF16, tag=f"wq{cc}", name=f"wq{cc}")
                nc.sync.dma_start(t[:], wq_d[cc])
                wq.append(t)
                t = constp.tile([128, C2], BF16, tag=f"wg{cc}", name=f"wg{cc}")
                nc.sync.dma_start(t[:], wg_d[cc])
                wg.append(t)
            for oc in range(2):
                t = constp.tile([C2, 128], BF16, tag=f"wo{oc}", name=f"wo{oc}")
                nc.sync.dma_start(t[:], wo_d[oc])
                wo.append(t)
            id_b = constp.tile([128, 128], BF16, tag="id_b", name="id_b")
            nc.sync.dma_start(id_b[:], id_d[:])
            ones = constp.tile([128, 128], BF16, tag="ones", name="ones")
            nc.gpsimd.memset(ones[:], 1.0)

            # ---- x: bf16 casting loads, resident for residual ----
            xb = [dict() for _ in range(BPC)]
            for b in range(BPC):
                for half in range(2):
                    for cc in range(2):
                        t = datap.tile([128, 2048], BF16, tag=f"xb{b}{cc}{half}",
                                       name=f"xb{b}_{cc}_{half}")
                        xb[b][(cc, half)] = t
            for b in range(BPC):
                for half in range(2):
                    for p0 in range(0, 2048, 512):
                        for cc in range(2):
                            src = slice(half * 2048 + p0, half * 2048 + p0 + 512)
                            nc.gpsimd.dma_start(
                                xb[b][(cc, half)][:, p0:p0 + 512],
                                x_d[b, cc * 128:(cc + 1) * 128, src])

            # ---- per-sample state ----
            st = []
            for b in range(BPC):
                s = dict(aps={}, at={})
                s["th2"] = datap.tile([64, N], BF16, tag=f"th2{b}",
                                      name=f"th2_{b}")
                s["ph2"] = datap.tile([64, M], BF16, tag=f"ph2{b}",
                                      name=f"ph2_{b}")
                s["gp"] = datap.tile([C2, M], BF16, tag=f"gp{b}", name=f"gp_{b}")
                # ets / psum partial tiles are 2-quarter rings along free axis
                s["ets"] = [datap.tile([128, 2048], BF16, tag=f"ets{b}{mc}",
                                       name=f"ets{b}_{mc}") for mc in range(MC)]
                s["psum4"] = [datap.tile([128, 2048], BF16, tag=f"ps4_{b}{j}",
                                         name=f"ps4_{b}_{j}") for j in range(4)]
                s["gts"] = [None] * MC
                st.append(s)

            # ---------------- emitters ----------------
            def proj_chunk(b, i):
                half, off = i // 4, (i % 4) * 512
                s = st[b]
                xs = [xb[b][(cc, half)][:, off:off + 512] for cc in range(2)]
                # separate 1-bank tiles in the unit ring: the big ring stays
                # dedicated to score pairs (avoids ACT-gated ring stalls)
                gd = psu.tile([128, 512], F32, tag="u", name=f"pg{b}_{i}")
                qd = psu.tile([128, 512], F32, tag="u", name=f"pq{b}_{i}")
                for cc in range(2):
                    nc.tensor.matmul(gd[:], wg[cc][:], xs[cc],
                                     start=(cc == 0), stop=(cc == 1))
                # q-proj rows: [th th ph ph] duplicated for quadrant packing
                for cc in range(2):
                    nc.tensor.matmul(qd[:], wq[cc][:], xs[cc],
                                     start=(cc == 0), stop=(cc == 1))
                sl = slice(i * 512, (i + 1) * 512)
                nc.scalar.copy(s["th2"][:, sl], qd[0:64, :])
                # fused 2x2 maxpools: single DVE reduce over (hb, two) axes
                msl = slice(i * 128, (i + 1) * 128)
                pv = qd[64:128, :].rearrange(
                    "p (h2 hb w2 two) -> p h2 w2 hb two", h2=4, hb=2, w2=32, two=2)
                nc.vector.tensor_reduce(
                    s["ph2"][:, msl].rearrange("p (h2 w2) -> p h2 w2", h2=4, w2=32),
                    pv, mybir.AxisListType.XY, AluOpType.max)
                gv = gd[:].rearrange(
                    "p (h2 hb w2 two) -> p h2 w2 hb two", h2=4, hb=2, w2=32, two=2)
                nc.vector.tensor_reduce(
                    s["gp"][:, msl].rearrange("p (h2 w2) -> p h2 w2", h2=4, w2=32),
                    gv, mybir.AxisListType.XY, AluOpType.max)

            def tp_chunk(b, mc):
                s = st[b]
                tp = psb.tile([128, 128], BF16, tag="big", name=f"tp{b}_{mc}")
                nc.tensor.transpose(tp[:], s["gp"][:, mc * 128:(mc + 1) * 128],
                                    id_b[:])
                gt = datap.tile([128, 128], BF16, tag=f"gt{b}{mc}",
                                name=f"gt{b}_{mc}")
                nc.vector.tensor_copy(gt[:], tp[:])
                s["gts"][mc] = gt

            def score_pair(b, qt, r):
                """Quadrant-packed scores for m-chunks (2r, 2r+1)."""
                s = st[b]
                ring = (qt % 2) * 1024
                qsl = slice(ring, ring + 1024)
                mca, mcb = 2 * r, 2 * r + 1
                spa = psb.tile([128, 1024], F32, tag="big", name=f"sa{b}_{qt}_{r}")
                spb = psb.tile([128, 1024], F32, tag="big", name=f"sb{b}_{qt}_{r}")
                for q, sp_t, mc in ((0, spa, mca), (32, spb, mcb)):
                    for hf in range(2):
                        nsl = slice(qt * 1024 + hf * 512,
                                    qt * 1024 + (hf + 1) * 512)
                        osl = slice(hf * 512, (hf + 1) * 512)
                        nc.tensor.matmul(sp_t[:, osl],
                                         s["ph2"][q:q + 32,
                                                  mc * 128:(mc + 1) * 128],
                                         s["th2"][q:q + 32, nsl],
                                         start=True, stop=True,
                                         tile_position=(q, 0))
                nc.scalar.activation(s["ets"][mca][:, qsl], spa[:], EXP)
                nc.scalar.activation(s["ets"][mcb][:, qsl], spb[:], EXP)
                # denominator partials: pairwise adds + in-place tree on DVE
                ps4 = s["psum4"]
                nc.vector.tensor_tensor(ps4[r][:, qsl], s["ets"][mca][:, qsl],
                                        s["ets"][mcb][:, qsl], AluOpType.add)
                if r == 3:
                    nc.vector.tensor_tensor(ps4[0][:, qsl], ps4[0][:, qsl],
                                            ps4[1][:, qsl], AluOpType.add)
                    nc.vector.tensor_tensor(ps4[2][:, qsl], ps4[2][:, qsl],
                                            ps4[3][:, qsl], AluOpType.add)
                    nc.vector.tensor_tensor(ps4[0][:, qsl], ps4[0][:, qsl],
                                            ps4[2][:, qsl], AluOpType.add)

            def unit_attn2(b, i0):
                s = st[b]
                ring = ((i0 // 2) % 2) * 1024
                rs = [slice(ring + (i % 2) * 512, ring + (i % 2) * 512 + 512)
                      for i in (i0, i0 + 1)]
                ap2 = []
                for i in (i0, i0 + 1):
                    aps = psu.tile([128, 512], F32, tag="u", name=f"aps{b}_{i}")
                    s["aps"][i] = aps
                    ap2.append(aps)
                for mc in range(MC):
                    for j in range(2):
                        nc.tensor.matmul(ap2[j][:], s["gts"][mc][:],
                                         s["ets"][mc][:, rs[j]],
                                         start=(mc == 0), stop=(mc == MC - 1),
                                         skip_group_check=True)

            def unit_den2(b, i0):
                s = st[b]
                ring = ((i0 // 2) % 2) * 1024
                for i in (i0, i0 + 1):
                    rsl = slice(ring + (i % 2) * 512, ring + (i % 2) * 512 + 512)
                    dps = psu.tile([128, 512], F32, tag="u", name=f"dps{b}_{i}")
                    nc.tensor.matmul(dps[:], ones[:], s["psum4"][0][:, rsl],
                                     start=True, stop=True)
                    rec = smallp.tile([128, 512], F32, tag="rec",
                                      name=f"rec{b}_{i}")
                    nc.vector.reciprocal_approx_fast(rec[:], dps[:])
                    at = smallp.tile([128, 512], BF16, tag="at", name=f"at{b}_{i}")
                    nc.vector.scalar_tensor_tensor(
                        at[:], s["aps"][i][:], 1.0, rec[:],
                        AluOpType.bypass, AluOpType.mult)
                    s["at"][i] = at

            def unit_out(b, i):
                s = st[b]
                nsl = slice(i * 512, (i + 1) * 512)
                half, off = i // 4, (i % 4) * 512
                for oc in range(2):
                    op = psu.tile([128, 512], F32, tag="u", name=f"op{b}_{i}_{oc}")
                    nc.tensor.matmul(op[:], id_b[:],
                                     xb[b][(oc, half)][:, off:off + 512],
                                     start=True, stop=False,
                                     skip_group_check=True)
                    nc.tensor.matmul(op[:], wo[oc][:], s["at"][i][:],
                                     start=False, stop=True,
                                     skip_group_check=True)
                    osb = outp.tile([128, 512], F32, tag="osb",
                                    name=f"osb{b}_{i}_{oc}")
                    if oc == 0:
                        nc.scalar.copy(osb[:], op[:])
                    else:
                        nc.vector.tensor_copy(osb[:], op[:])
                    nc.sync.dma_start(out_d[b, oc * 128:(oc + 1) * 128, nsl],
                                      osb[:])

            # ---------------- emission schedule ----------------
            # Phase order interleaves the two samples and overlaps sample 1's
            # projections/transposes into sample 0's early score phases; units
            # trail their quarter by one phase (FIFO).
            pending = []

            def pop(k):
                for _ in range(k):
                    if pending:
                        pending.pop(0)()

            oq = []

            def queue_units(b, qt):
                # attn/den trail one phase; out units trail two (their
                # normalized inputs are then long-ready on DVE)
                i0 = 2 * qt
                pending.append(lambda: unit_attn2(b, i0))
                pending.append(lambda: unit_den2(b, i0))
                pending.extend(oq)
                oq.clear()
                oq.append(lambda: unit_out(b, i0))
                oq.append(lambda: unit_out(b, i0 + 1))

            # opening: proj.b0 with qt0.b0's score pairs interleaved (pair r
            # needs only proj chunks <= 2r+1), transposes at the end
            for r in range(4):
                proj_chunk(0, 2 * r)
                proj_chunk(0, 2 * r + 1)
                score_pair(0, 0, r)
            for mc in range(MC):
                tp_chunk(0, mc)
            queue_units(0, 0)

            # filler work per phase: (emitted after round r as PE filler)
            fillers = {
                (0, 1): [lambda i=i: proj_chunk(1, i) for i in range(3)],
                (0, 2): [lambda i=i: proj_chunk(1, i) for i in range(3, 6)],
                (1, 0): [lambda i=i: proj_chunk(1, i) for i in range(6, MC)]
                        + [lambda mc=mc: tp_chunk(1, mc) for mc in range(MC)],
            }
            phase_list = [(0, 1), (0, 2), (1, 0), (0, 3), (1, 1),
                          (1, 2), (1, 3)]
            for b, qt in phase_list:
                fill = fillers.get((b, qt), [])
                fi = 0
                for r in range(4):
                    score_pair(b, qt, r)
                    take = (len(fill) * (r + 1)) // 4 - fi
                    for _ in range(take):
                        fill[fi]()
                        fi += 1
                    if r == 1:
                        pop(1)      # attn pair
                    elif r == 2:
                        pop(1)      # den pair
                    elif r == 3:
                        pop(2)      # out units
                queue_units(b, qt)
            pending.extend(oq)
            oq.clear()
            while pending:
                pop(1)

    nc.compile()
    return nc


_NC_CACHE = None


def _get_nc():
    global _NC_CACHE
    if _NC_CACHE is None:
        _NC_CACHE = build_kernel()
    return _NC_CACHE


def prep_inputs(x, w_theta, w_phi, w_g, w_o, gamma):
    """Host-side prep: shard x over 8 cores; transpose/scale/pack weights."""
    x = np.asarray(x, dtype=np.float32).reshape(B, C, N)
    w_theta = np.asarray(w_theta, dtype=np.float32)
    w_phi = np.asarray(w_phi, dtype=np.float32)
    w_g = np.asarray(w_g, dtype=np.float32)
    w_o = np.asarray(w_o, dtype=np.float32)
    gamma = np.float32(gamma)

    # [th th ph ph] along the output dim for quadrant-packed scores
    import ml_dtypes
    bf16 = ml_dtypes.bfloat16
    wqT = np.concatenate([w_theta.T, w_theta.T, w_phi.T, w_phi.T], axis=1)
    wq = np.ascontiguousarray(wqT.reshape(2, 128, 128)).astype(bf16)
    wgq = np.ascontiguousarray(w_g.T.reshape(2, 128, C2)).astype(bf16)
    woT = (gamma * w_o).T                                     # [128, 256]
    wo = np.ascontiguousarray(
        woT.reshape(C2, 2, 128).transpose(1, 0, 2)).astype(bf16)
    ident = np.eye(128, dtype=np.float32).astype(bf16)

    in_maps = []
    for core in range(NCORES):
        shard = np.ascontiguousarray(x[core * BPC:(core + 1) * BPC])
        in_maps.append({"x": shard, "wq": wq, "wg": wgq, "wo": wo,
                        "ident": ident})
    return in_maps


def run(inputs, trace=False, **kw):
    nc = _get_nc()
    in_maps = prep_inputs(**inputs)
    res = run_bass_kernel_spmd(nc, in_maps, core_ids=list(range(NCORES)),
                               trace=trace, **kw)
    outs = [res.results[i]["out"] for i in range(NCORES)]
    full = np.concatenate(outs, axis=0).reshape(B, C, H, W).astype(np.float32)
    return full, res


def kernel(**inputs):
    full, _ = run(inputs, trace=False)
    return full
